# revision 10
# baseline (speedup 1.0000x reference)
"""Additive (Bahdanau) attention on 8 TRN2 NeuronCores.

Reference computation (per batch b of 8, one NeuronCore each):
    qp[q,h] = Q[q,:] @ Wq.T + bq          (Lq=256, D=512, H=256)
    kp[k,h] = K[k,:] @ Wk.T + bk + b_param
    E[q,k]  = sum_h v[h] * tanh(qp[q,h] + kp[k,h])
    E += -1e30 where mask[k]==0
    A = softmax_k(exp(E))                 (no max-subtraction needed; |E| small)
    ctx = A @ V

Device mapping (per core):
  - projections: PE matmuls in float32r, contraction over d (host passes
    pre-transposed QT/KT/WqT/WkT so d sits on partitions)
  - broadcast add qp[:,q] + kp: DVE tensor_scalar (bf16, 4x mode), batched
    into [128, 32*256] tiles
  - tanh: ACT, one op per (group, h-chunk) over [128, 8192] (in-place)
  - v-reduction: PE, lhsT = tanh block [128h, 128k] (weights), rhs = v[128,1]
    -> energies^T columns [k, q] accumulated in PSUM over h-chunks
  - mask+exp: ACT, PSUM src, per-partition bias = mask column
  - softmax sums: PE, lhsT = exp^T tile, rhs = ones -> [q,1] sums
  - context: PE, lhsT = exp^T tile, rhs = V(bf16); 1/sum folded into the
    PSUM->SBUF copy as per-partition scale
  - attention output: PE transpose of exp^T tiles, scaled on copy
"""

import os
import numpy as np

B, LQ, LK = 8, 256, 256
D, H = 512, 256
HC, KC, QC, DC = 2, 2, 2, 4     # 128-chunks of h, k, q, d
GQ = 32                         # q's per group
GPC = 128 // GQ                 # groups per q-chunk

_CACHE: dict = {}


def _build_nc():
    import concourse.bacc as bacc
    import concourse.tile as tile
    from concourse import mybir

    f32 = mybir.dt.float32
    bf16 = mybir.dt.bfloat16
    i32 = mybir.dt.int32
    AF = mybir.ActivationFunctionType
    ALU = mybir.AluOpType

    nc = bacc.Bacc("TRN2", target_bir_lowering=False)

    qt = nc.declare_dram_parameter("qt", [D, LQ], f32, isOutput=False)
    kt = nc.declare_dram_parameter("kt", [D, LK], f32, isOutput=False)
    vv = nc.declare_dram_parameter("v", [LK, D], f32, isOutput=False)
    wqt = nc.declare_dram_parameter("wqt", [D, H], f32, isOutput=False)
    wkt = nc.declare_dram_parameter("wkt", [D, H], f32, isOutput=False)
    bq2 = nc.declare_dram_parameter("bq2", [128, HC], f32, isOutput=False)
    bk2 = nc.declare_dram_parameter("bk2", [128, HC], f32, isOutput=False)
    bp2 = nc.declare_dram_parameter("bp2", [128, HC], f32, isOutput=False)
    vp2 = nc.declare_dram_parameter("vp2", [128, HC], f32, isOutput=False)
    msk = nc.declare_dram_parameter("mask2", [1, LK], i32, isOutput=False)
    idn = nc.declare_dram_parameter("ident", [128, 128], f32, isOutput=False)
    out_ctx = nc.declare_dram_parameter("out_ctx", [LQ, D], f32, isOutput=True)
    out_attn = nc.declare_dram_parameter("out_attn", [LQ, LK], f32, isOutput=True)

    with tile.TileContext(nc) as tc:
        with (
            tc.tile_pool(name="const", bufs=1) as cpool,
            tc.tile_pool(name="stage", bufs=2) as spool,
            tc.tile_pool(name="arg", bufs=3) as apool,
            tc.tile_pool(name="exp", bufs=8) as epool,
            tc.tile_pool(name="outp", bufs=2) as opool,
            tc.tile_pool(name="psA", bufs=4, space="PSUM") as psA,
            tc.tile_pool(name="psB", bufs=4, space="PSUM") as psB,
        ):
            # ---- load inputs; convert projection operands to bf16 ----
            qt_sb, kt_sb, wqt_sb, wkt_sb = [], [], [], []
            for src, dst, w in ((qt, qt_sb, LQ), (kt, kt_sb, LK),
                                (wqt, wqt_sb, H), (wkt, wkt_sb, H)):
                for dc in range(DC):
                    st = spool.tile([128, w], f32, tag="lstage",
                                    name=f"ls_{src.name}{dc}")
                    nc.sync.dma_start(st, src[dc * 128:(dc + 1) * 128, :])
                    t = cpool.tile([128, w], bf16,
                                   tag=f"{src.name}bf{dc}",
                                   name=f"{src.name}bf{dc}")
                    nc.vector.tensor_copy(t, st)
                    dst.append(t)

            v_bf = []
            for kc in range(KC):
                vf = spool.tile([128, D], f32, tag="vstage")
                nc.sync.dma_start(vf, vv[kc * 128:(kc + 1) * 128, :])
                vb = cpool.tile([128, D], bf16, tag=f"v{kc}")
                nc.vector.tensor_copy(vb, vf)
                v_bf.append(vb)

            bq_sb = cpool.tile([128, HC], f32, tag="bq")
            nc.sync.dma_start(bq_sb, bq2[:])
            bk_sb = cpool.tile([128, HC], f32, tag="bk")
            nc.sync.dma_start(bk_sb, bk2[:])
            bp_sb = cpool.tile([128, HC], f32, tag="bp")
            nc.sync.dma_start(bp_sb, bp2[:])
            vp_sb = cpool.tile([128, HC], f32, tag="vp")
            nc.sync.dma_start(vp_sb, vp2[:])
            vp_bf = cpool.tile([128, HC], bf16, tag="vpbf")
            nc.vector.tensor_copy(vp_bf, vp_sb)

            bkb = cpool.tile([128, HC], f32, tag="bkb")
            nc.vector.tensor_add(bkb, bk_sb, bp_sb)

            msk_sb = cpool.tile([1, LK], i32, tag="msk")
            nc.sync.dma_start(msk_sb, msk[:])
            mask_bf = cpool.tile([1, LK], bf16, tag="maskbf")
            nc.vector.tensor_scalar(mask_bf, msk_sb, 0, -1e30, ALU.is_equal,
                                    ALU.mult)
            ones_row = cpool.tile([1, 128], bf16, tag="onesrow")
            nc.vector.memset(ones_row, 1.0)

            idf = spool.tile([128, 128], f32, tag="idstage")
            nc.sync.dma_start(idf, idn[:])
            id_bf = cpool.tile([128, 128], bf16, tag="idbf")
            nc.vector.tensor_copy(id_bf, idf)

            ones_col = cpool.tile([128, 1], bf16, tag="ones")
            nc.vector.memset(ones_col, 1.0)

            # ---- projections: qp[h,q], kp[h,k] (h on partitions) ----
            qp_sb, kp_bf = [], []
            for hc in range(HC):
                pq = psA.tile([128, LQ], f32, tag="ps")
                for dc in range(DC):
                    nc.tensor.matmul(
                        pq,
                        lhsT=wqt_sb[dc][:, hc * 128:(hc + 1) * 128],
                        rhs=qt_sb[dc],
                        start=(dc == 0), stop=(dc == DC - 1))
                qp = cpool.tile([128, LQ], f32, tag=f"qp{hc}")
                nc.vector.tensor_scalar_add(qp, pq, bq_sb[:, hc:hc + 1])
                qp_sb.append(qp)

                pk = psA.tile([128, LK], f32, tag="ps")
                for dc in range(DC):
                    nc.tensor.matmul(
                        pk,
                        lhsT=wkt_sb[dc][:, hc * 128:(hc + 1) * 128],
                        rhs=kt_sb[dc],
                        start=(dc == 0), stop=(dc == DC - 1))
                kp = cpool.tile([128, LK], bf16, tag=f"kp{hc}")
                nc.vector.tensor_scalar_add(kp, pk, bkb[:, hc:hc + 1])
                kp_bf.append(kp)

            # ---- main loop over q-chunks ----
            for qc in range(QC):
                # energies^T tiles [k, q] per k-chunk, accumulated over h
                et = [psA.tile([128, 128], f32, tag="ps", name=f"et{kc}")
                      for kc in range(KC)]
                # init: whole-bank start zeroes the region and deposits the
                # mask penalty (-1e30 where mask==0) along partitions (k)
                for kc in range(KC):
                    nc.tensor.matmul(
                        et[kc],
                        lhsT=mask_bf[:, kc * 128:(kc + 1) * 128],
                        rhs=ones_row,
                        start=True, stop=False)

                for g in range(GPC):
                    arg = apool.tile([128, HC, GQ, LK], bf16, tag="arg")
                    for hc in range(HC):
                        for j in range(GQ):
                            qi = qc * 128 + g * GQ + j
                            nc.vector.tensor_scalar_add(
                                arg[:, hc, j, :], kp_bf[hc],
                                qp_sb[hc][:, qi:qi + 1])
                    for hc in range(HC):
                        nc.scalar.activation(arg[:, hc], arg[:, hc], AF.Tanh)
                    for hc in range(HC):
                        for j in range(GQ):
                            qj = g * GQ + j
                            for kc in range(KC):
                                last = (g == GPC - 1 and hc == HC - 1
                                        and j == GQ - 1)
                                nc.tensor.matmul(
                                    et[kc][:, qj:qj + 1],
                                    lhsT=arg[:, hc, j,
                                             kc * 128:(kc + 1) * 128],
                                    rhs=vp_bf[:, hc:hc + 1],
                                    start=False, stop=last)

                # mask + exp (PSUM -> SBUF bf16)
                expts = []
                for kc in range(KC):
                    e = epool.tile([128, 128], bf16, tag="exp", name=f"exp{kc}")
                    nc.scalar.activation(e, et[kc], AF.Exp)
                    expts.append(e)

                # softmax denominators: sums[q,1] = sum_k exp^T[k,q]
                sums = psB.tile([128, 1], f32, tag="misc")
                for kc in range(KC):
                    nc.tensor.matmul(sums, lhsT=expts[kc], rhs=ones_col,
                                     start=(kc == 0), stop=(kc == KC - 1))
                recip = opool.tile([128, 1], f32, tag="recip")
                nc.vector.reciprocal(recip, sums)

                # context = (exp^T).T @ V, scaled by recip on copy-out
                ctxp = psB.tile([128, D], f32, tag="misc")
                for kc in range(KC):
                    nc.tensor.matmul(ctxp, lhsT=expts[kc], rhs=v_bf[kc],
                                     start=(kc == 0), stop=(kc == KC - 1))
                ctx_sb = opool.tile([128, D], f32, tag="ctx")
                nc.vector.tensor_scalar_mul(ctx_sb, ctxp, recip)
                nc.sync.dma_start(out_ctx[qc * 128:(qc + 1) * 128, :], ctx_sb)

                # attention weights: transpose exp^T back to [q,k], scale
                attn_sb = opool.tile([128, LK], f32, tag="attn")
                for kc in range(KC):
                    tp = psB.tile([128, 128], bf16, tag="misc")
                    nc.tensor.transpose(tp, expts[kc], id_bf)
                    nc.vector.tensor_scalar_mul(
                        attn_sb[:, kc * 128:(kc + 1) * 128], tp, recip)
                nc.sync.dma_start(out_attn[qc * 128:(qc + 1) * 128, :],
                                  attn_sb)

    nc.compile()
    return nc


def _get_nc():
    if "nc" not in _CACHE:
        _CACHE["nc"] = _build_nc()
    return _CACHE["nc"]


def make_in_maps(Q, K, V, mask, Wq, bq, Wk, bk, v_param, b_param):
    Q = np.asarray(Q, dtype=np.float32)
    K = np.asarray(K, dtype=np.float32)
    V = np.asarray(V, dtype=np.float32)
    mask = np.asarray(mask, dtype=np.int32)
    Wq = np.asarray(Wq, dtype=np.float32)
    Wk = np.asarray(Wk, dtype=np.float32)
    bq = np.asarray(bq, dtype=np.float32)
    bk = np.asarray(bk, dtype=np.float32)
    v_param = np.asarray(v_param, dtype=np.float32)
    b_param = np.asarray(b_param, dtype=np.float32)

    wqt = np.ascontiguousarray(Wq.T)            # [D, H]
    wkt = np.ascontiguousarray(Wk.T)
    bq2 = np.ascontiguousarray(bq.reshape(HC, 128).T)
    bk2 = np.ascontiguousarray(bk.reshape(HC, 128).T)
    bp2 = np.ascontiguousarray(b_param.reshape(HC, 128).T)
    vp2 = np.ascontiguousarray(v_param.reshape(HC, 128).T)
    ident = np.eye(128, dtype=np.float32)

    in_maps = []
    for b in range(B):
        in_maps.append({
            "qt": np.ascontiguousarray(Q[b].T),
            "kt": np.ascontiguousarray(K[b].T),
            "v": np.ascontiguousarray(V[b]),
            "wqt": wqt,
            "wkt": wkt,
            "bq2": bq2,
            "bk2": bk2,
            "bp2": bp2,
            "vp2": vp2,
            "mask2": np.ascontiguousarray(mask[b].reshape(1, LK)),
            "ident": ident,
        })
    return in_maps


def kernel(Q, K, V, mask, Wq, bq, Wk, bk, v_param, b_param,
           _trace=False):
    from concourse.bass_utils import run_bass_kernel_spmd

    nc = _get_nc()
    in_maps = make_in_maps(Q, K, V, mask, Wq, bq, Wk, bk, v_param, b_param)
    res = run_bass_kernel_spmd(nc, in_maps, core_ids=list(range(B)),
                               trace=_trace)
    outs = res.results
    context = np.stack([np.asarray(outs[b]["out_ctx"]) for b in range(B)])
    attn = np.stack([np.asarray(outs[b]["out_attn"]) for b in range(B)])
    if _trace:
        return (context, attn), res
    return context, attn


# revision 13
# speedup vs baseline: 2.3904x; 2.3904x over previous
"""Additive (Bahdanau) attention on 8 TRN2 NeuronCores — sine-series kernel.

Per batch b (one NeuronCore each):
    qp[q,h] = Q[q,:] @ Wq.T + bq
    kp[k,h] = K[k,:] @ Wk.T + bk + b_param
    E[q,k]  = sum_h v[h] * tanh(qp[q,h] + kp[k,h])
    A = softmax_k(E + mask_penalty); ctx = A @ V

Key trick: tanh(x) ~ sum_t g_t sin(w_t x) (least-squares sine series,
w_t = t*pi/L).  sin(w(q+k)) separates:
    sin(wq)cos(wk) + cos(wq)sin(wk),  cos(z) = 1 - 2 sin^2(z/2)
so with s = sin(wx), u = sin^2(wx/2) per side:
    E = sum_t g_t [ s_q + s_k - 2 s_q u_k - 2 u_q s_k ]
The pure-q term is softmax-invariant and is dropped.  E becomes ONE PE
matmul with contraction over (3 blocks per t) x h:
    blocks per t: (s_q | -2 g v u_k), (u_q | -2 g v s_k), (-0.5 | -2 g v s_k)

Engine mapping per core:
  - projections (PE, bf16), psum->sbuf copy folds biases (DVE)
  - per (t, side): y = x * w/(2pi) (DVE ts), r = round(y) via the
    +-1.5*2^23 magic trick (one fused DVE ts), f = y - r (DVE TT),
    s = ACT Sin(f, scale=2pi), s' = ACT Sin(f, scale=pi),
    u = ACT Square(s'); k-side weighted to bf16 by DVE ts (v col, -2g)
  - energies^T [k, q] accumulate in PSUM over 3T*2 chunk matmuls; the
    first matmul deposits the mask penalty and zeroes the bank
  - exp (ACT, PSUM src) -> bf16; sums via exp-as-weights matmul with a
    ones column; context = exp^T.T @ V with 1/sum as per-partition scale
    on the psum->sbuf copy; attention out via PE transpose + scale.
"""

import numpy as np

B, LQ, LK = 8, 256, 256
D, H = 512, 256
HC, KC, QC, DC = 2, 2, 2, 4
T_FREQ = 10
L_PERIOD = 7.0
RIDGE = 1e-7
XMAX = 5.2

_CACHE: dict = {}


def _fit_sine(T=T_FREQ, L=L_PERIOD, ridge=RIDGE, xmax=XMAX,
              nsamp=200000, seed=0):
    rng = np.random.default_rng(seed)
    xs = np.concatenate([rng.normal(0, 0.85, nsamp),
                         np.linspace(-xmax, xmax, 4001)])
    w = np.concatenate([np.full(nsamp, 1.0),
                        np.full(4001, nsamp / 4001 * 0.05)])
    om = np.arange(1, T + 1) * np.pi / L
    A = np.sin(xs[:, None] * om[None, :])
    Wm = np.sqrt(w)[:, None]
    AtA = (A * Wm).T @ (A * Wm) + ridge * nsamp * np.eye(T)
    Atb = (A * Wm).T @ (np.tanh(xs) * Wm[:, 0])
    g = np.linalg.solve(AtA, Atb)
    return om, g


def _build_nc():
    import concourse.bacc as bacc
    import concourse.tile as tile
    from concourse import mybir

    f32 = mybir.dt.float32
    bf16 = mybir.dt.bfloat16
    i32 = mybir.dt.int32
    AF = mybir.ActivationFunctionType
    ALU = mybir.AluOpType

    om, gam = _fit_sine()
    MAGIC = float(1.5 * 2 ** 23)
    TWO_PI = float(2 * np.pi)
    PI = float(np.pi)

    nc = bacc.Bacc("TRN2", target_bir_lowering=False)

    qt = nc.declare_dram_parameter("qt", [D, LQ], f32, isOutput=False)
    kt = nc.declare_dram_parameter("kt", [D, LK], f32, isOutput=False)
    vv = nc.declare_dram_parameter("v", [LK, D], f32, isOutput=False)
    wqt = nc.declare_dram_parameter("wqt", [D, H], f32, isOutput=False)
    wkt = nc.declare_dram_parameter("wkt", [D, H], f32, isOutput=False)
    bq2 = nc.declare_dram_parameter("bq2", [128, HC], f32, isOutput=False)
    bk2 = nc.declare_dram_parameter("bk2", [128, HC], f32, isOutput=False)
    bp2 = nc.declare_dram_parameter("bp2", [128, HC], f32, isOutput=False)
    vp2 = nc.declare_dram_parameter("vp2", [128, HC], f32, isOutput=False)
    msk = nc.declare_dram_parameter("mask2", [1, LK], i32, isOutput=False)
    idn = nc.declare_dram_parameter("ident", [128, 128], f32, isOutput=False)
    out_ctx = nc.declare_dram_parameter("out_ctx", [LQ, D], f32, isOutput=True)
    out_attn = nc.declare_dram_parameter("out_attn", [LQ, LK], f32,
                                         isOutput=True)

    with tile.TileContext(nc) as tc:
        with (
            tc.tile_pool(name="const", bufs=1) as cpool,
            tc.tile_pool(name="stage", bufs=3) as spool,
            tc.tile_pool(name="feat", bufs=1) as fpool,
            tc.tile_pool(name="ftmp", bufs=4) as tpool,
            tc.tile_pool(name="exp", bufs=2) as epool,
            tc.tile_pool(name="outp", bufs=2) as opool,
            tc.tile_pool(name="psA", bufs=4, space="PSUM") as psA,
            tc.tile_pool(name="psB", bufs=4, space="PSUM") as psB,
        ):
            # ---- loads; projection operands to bf16 ----
            qt_sb, kt_sb, wqt_sb, wkt_sb = [], [], [], []
            for src, dst, w in ((kt, kt_sb, LK), (wkt, wkt_sb, H),
                                (qt, qt_sb, LQ), (wqt, wqt_sb, H)):
                for dc in range(DC):
                    st = spool.tile([128, w], f32, tag="lstage",
                                    name=f"ls_{src.name}{dc}")
                    nc.sync.dma_start(st, src[dc * 128:(dc + 1) * 128, :])
                    t = cpool.tile([128, w], bf16, tag=f"{src.name}bf{dc}",
                                   name=f"{src.name}bf{dc}")
                    nc.vector.tensor_copy(t, st)
                    dst.append(t)

            v_bf = []
            for kc in range(KC):
                vf = spool.tile([128, D], f32, tag="vstage")
                nc.sync.dma_start(vf, vv[kc * 128:(kc + 1) * 128, :])
                vb = cpool.tile([128, D], bf16, tag=f"v{kc}")
                nc.vector.tensor_copy(vb, vf)
                v_bf.append(vb)

            bq_sb = cpool.tile([128, HC], f32, tag="bq")
            nc.sync.dma_start(bq_sb, bq2[:])
            bk_sb = cpool.tile([128, HC], f32, tag="bk")
            nc.sync.dma_start(bk_sb, bk2[:])
            bp_sb = cpool.tile([128, HC], f32, tag="bp")
            nc.sync.dma_start(bp_sb, bp2[:])
            vp_sb = cpool.tile([128, HC], f32, tag="vp")
            nc.sync.dma_start(vp_sb, vp2[:])
            bkb = cpool.tile([128, HC], f32, tag="bkb")
            nc.vector.tensor_add(bkb, bk_sb, bp_sb)

            msk_sb = cpool.tile([1, LK], i32, tag="msk")
            nc.sync.dma_start(msk_sb, msk[:])
            mask_bf = cpool.tile([1, LK], bf16, tag="maskbf")
            nc.vector.tensor_scalar(mask_bf, msk_sb, 0, -1e30, ALU.is_equal,
                                    ALU.mult)
            ones_row = cpool.tile([1, LQ], bf16, tag="onesrow")
            nc.vector.memset(ones_row, 1.0)
            # q-side "ones" feature carries the -0.5 factor
            halfneg = cpool.tile([128, LQ], bf16, tag="halfneg")
            nc.vector.memset(halfneg, -0.5)

            idf = spool.tile([128, 128], f32, tag="idstage")
            nc.sync.dma_start(idf, idn[:])
            id_bf = cpool.tile([128, 128], bf16, tag="idbf")
            nc.vector.tensor_copy(id_bf, idf)
            ones_col = cpool.tile([128, 1], bf16, tag="ones")
            nc.vector.memset(ones_col, 1.0)

            # ---- projections: qp[h,(hc,q)], kp[h,(hc,k)] in f32 ----
            qp_cat = cpool.tile([128, HC, LQ], f32, tag="qpcat")
            kp_cat = cpool.tile([128, HC, LK], f32, tag="kpcat")
            for hc in range(HC):
                pk = psA.tile([128, LK], f32, tag="ps")
                for dc in range(DC):
                    nc.tensor.matmul(
                        pk, lhsT=wkt_sb[dc][:, hc * 128:(hc + 1) * 128],
                        rhs=kt_sb[dc], start=(dc == 0), stop=(dc == DC - 1))
                nc.vector.tensor_scalar_add(kp_cat[:, hc, :], pk,
                                            bkb[:, hc:hc + 1])
                pq = psA.tile([128, LQ], f32, tag="ps")
                for dc in range(DC):
                    nc.tensor.matmul(
                        pq, lhsT=wqt_sb[dc][:, hc * 128:(hc + 1) * 128],
                        rhs=qt_sb[dc], start=(dc == 0), stop=(dc == DC - 1))
                nc.vector.tensor_scalar_add(qp_cat[:, hc, :], pq,
                                            bq_sb[:, hc:hc + 1])

            # ---- energies^T psum tiles [k, q], one per k-chunk ----
            et = [psA.tile([128, LQ], f32, tag="ps", name=f"et{kc}")
                  for kc in range(KC)]
            for kc in range(KC):
                nc.tensor.matmul(et[kc],
                                 lhsT=mask_bf[:, kc * 128:(kc + 1) * 128],
                                 rhs=ones_row, start=True, stop=False)

            # ---- per-frequency features + energy matmuls ----
            n_mm = [1, 1]   # per-kc matmul count (mask mm counted)
            total_mm = 1 + T_FREQ * 3 * HC
            for t in range(T_FREQ):
                sc_y = float(om[t] / TWO_PI)
                g = float(gam[t])
                feats = {}
                for side, xcat in (("q", qp_cat), ("k", kp_cat)):
                    y = tpool.tile([128, HC, 256], f32, tag="y",
                                   name=f"y{side}{t}")
                    nc.vector.tensor_scalar(y, xcat, sc_y, None, ALU.mult)
                    r = tpool.tile([128, HC, 256], f32, tag="r",
                                   name=f"r{side}{t}")
                    nc.vector.tensor_scalar(r, y, MAGIC, MAGIC, ALU.add,
                                            ALU.subtract)
                    f = tpool.tile([128, HC, 256], f32, tag="f",
                                   name=f"f{side}{t}")
                    nc.vector.tensor_sub(f, y, r)
                    if side == "q":
                        s = fpool.tile([128, HC, 256], bf16, tag=f"sq{t}",
                                       name=f"sq{t}")
                        nc.scalar.activation(s, f, AF.Sin, scale=TWO_PI)
                        sp = tpool.tile([128, HC, 256], f32, tag="sp",
                                        name=f"spq{t}")
                        nc.scalar.activation(sp, f, AF.Sin, scale=PI)
                        u = fpool.tile([128, HC, 256], bf16, tag=f"uq{t}",
                                       name=f"uq{t}")
                        nc.scalar.activation(u, sp, AF.Square)
                        feats["sq"], feats["uq"] = s, u
                    else:
                        s = tpool.tile([128, HC, 256], f32, tag="sk",
                                       name=f"sk{t}")
                        nc.scalar.activation(s, f, AF.Sin, scale=TWO_PI)
                        sp = tpool.tile([128, HC, 256], f32, tag="sp",
                                        name=f"spk{t}")
                        nc.scalar.activation(sp, f, AF.Sin, scale=PI)
                        u = tpool.tile([128, HC, 256], f32, tag="uk",
                                       name=f"uk{t}")
                        nc.scalar.activation(u, sp, AF.Square)
                        # weighted bf16: W_s = -2 g v s_k, W_u = -2 g v u_k
                        ws = fpool.tile([128, HC, 256], bf16, tag=f"ws{t}",
                                        name=f"ws{t}")
                        wu = fpool.tile([128, HC, 256], bf16, tag=f"wu{t}",
                                        name=f"wu{t}")
                        for hc in range(HC):
                            nc.vector.tensor_scalar(
                                ws[:, hc, :], s[:, hc, :],
                                vp_sb[:, hc:hc + 1], -2.0 * g,
                                ALU.mult, ALU.mult)
                            nc.vector.tensor_scalar(
                                wu[:, hc, :], u[:, hc, :],
                                vp_sb[:, hc:hc + 1], -2.0 * g,
                                ALU.mult, ALU.mult)
                        feats["ws"], feats["wu"] = ws, wu

                pairs = ((feats["wu"], feats["sq"]),
                         (feats["ws"], feats["uq"]),
                         (feats["ws"], halfneg))
                for kc in range(KC):
                    for wf, uf in pairs:
                        for hc in range(HC):
                            n_mm[kc] += 1
                            rhs = (uf[:, hc, :] if uf is not halfneg
                                   else halfneg)
                            nc.tensor.matmul(
                                et[kc],
                                lhsT=wf[:, hc, kc * 128:(kc + 1) * 128],
                                rhs=rhs,
                                start=False,
                                stop=(n_mm[kc] == total_mm))

            # ---- softmax + context + attention out ----
            expts = []
            for kc in range(KC):
                e = epool.tile([128, LQ], bf16, tag="exp", name=f"exp{kc}")
                nc.scalar.activation(e, et[kc], AF.Exp)
                expts.append(e)

            for qc in range(QC):
                sums = psB.tile([128, 1], f32, tag="misc", name=f"sums{qc}")
                for kc in range(KC):
                    nc.tensor.matmul(
                        sums, lhsT=expts[kc][:, qc * 128:(qc + 1) * 128],
                        rhs=ones_col, start=(kc == 0), stop=(kc == KC - 1))
                recip = opool.tile([128, 1], f32, tag="recip",
                                   name=f"recip{qc}")
                nc.vector.reciprocal(recip, sums)

                ctxp = psB.tile([128, D], f32, tag="misc", name=f"ctxp{qc}")
                for kc in range(KC):
                    nc.tensor.matmul(
                        ctxp, lhsT=expts[kc][:, qc * 128:(qc + 1) * 128],
                        rhs=v_bf[kc], start=(kc == 0), stop=(kc == KC - 1))
                ctx_sb = opool.tile([128, D], f32, tag="ctx",
                                    name=f"ctx{qc}")
                nc.vector.tensor_scalar_mul(ctx_sb, ctxp, recip)
                nc.sync.dma_start(out_ctx[qc * 128:(qc + 1) * 128, :], ctx_sb)

                attn_sb = opool.tile([128, LK], f32, tag="attn",
                                     name=f"attn{qc}")
                for kc in range(KC):
                    tp = psB.tile([128, 128], bf16, tag="misc",
                                  name=f"tp{qc}{kc}")
                    nc.tensor.transpose(
                        tp, expts[kc][:, qc * 128:(qc + 1) * 128], id_bf)
                    nc.vector.tensor_scalar_mul(
                        attn_sb[:, kc * 128:(kc + 1) * 128], tp, recip)
                nc.sync.dma_start(out_attn[qc * 128:(qc + 1) * 128, :],
                                  attn_sb)

    nc.compile()
    return nc


def _get_nc():
    if "nc" not in _CACHE:
        _CACHE["nc"] = _build_nc()
    return _CACHE["nc"]


def make_in_maps(Q, K, V, mask, Wq, bq, Wk, bk, v_param, b_param):
    Q = np.asarray(Q, dtype=np.float32)
    K = np.asarray(K, dtype=np.float32)
    V = np.asarray(V, dtype=np.float32)
    mask = np.asarray(mask, dtype=np.int32)
    Wq = np.asarray(Wq, dtype=np.float32)
    Wk = np.asarray(Wk, dtype=np.float32)
    bq = np.asarray(bq, dtype=np.float32)
    bk = np.asarray(bk, dtype=np.float32)
    v_param = np.asarray(v_param, dtype=np.float32)
    b_param = np.asarray(b_param, dtype=np.float32)

    wqt = np.ascontiguousarray(Wq.T)
    wkt = np.ascontiguousarray(Wk.T)
    bq2 = np.ascontiguousarray(bq.reshape(HC, 128).T)
    bk2 = np.ascontiguousarray(bk.reshape(HC, 128).T)
    bp2 = np.ascontiguousarray(b_param.reshape(HC, 128).T)
    vp2 = np.ascontiguousarray(v_param.reshape(HC, 128).T)
    ident = np.eye(128, dtype=np.float32)

    in_maps = []
    for b in range(B):
        in_maps.append({
            "qt": np.ascontiguousarray(Q[b].T),
            "kt": np.ascontiguousarray(K[b].T),
            "v": np.ascontiguousarray(V[b]),
            "wqt": wqt,
            "wkt": wkt,
            "bq2": bq2,
            "bk2": bk2,
            "bp2": bp2,
            "vp2": vp2,
            "mask2": np.ascontiguousarray(mask[b].reshape(1, LK)),
            "ident": ident,
        })
    return in_maps


def kernel(Q, K, V, mask, Wq, bq, Wk, bk, v_param, b_param, _trace=False):
    from concourse.bass_utils import run_bass_kernel_spmd

    nc = _get_nc()
    in_maps = make_in_maps(Q, K, V, mask, Wq, bq, Wk, bk, v_param, b_param)
    res = run_bass_kernel_spmd(nc, in_maps, core_ids=list(range(B)),
                               trace=_trace)
    outs = res.results
    context = np.stack([np.asarray(outs[b]["out_ctx"]) for b in range(B)])
    attn = np.stack([np.asarray(outs[b]["out_attn"]) for b in range(B)])
    if _trace:
        return (context, attn), res
    return context, attn


# revision 16
# speedup vs baseline: 2.9431x; 1.2312x over previous
"""Additive (Bahdanau) attention on 8 TRN2 NeuronCores — sine-series kernel.

Per batch b (one NeuronCore each):
    qp[q,h] = Q[q,:] @ Wq.T + bq
    kp[k,h] = K[k,:] @ Wk.T + bk + b_param
    E[q,k]  = sum_h v[h] * tanh(qp[q,h] + kp[k,h])
    A = softmax_k(E + mask_penalty); ctx = A @ V

Key trick: tanh(x) ~ sum_t g_t sin(w_t x) (least-squares sine series,
w_t = t*pi/L).  sin(w(q+k)) separates:
    sin(wq)cos(wk) + cos(wq)sin(wk),  cos(z) = 1 - 2 sin^2(z/2)
so with s = sin(wx), u = sin^2(wx/2) per side:
    E = sum_t g_t [ s_q + s_k - 2 s_q u_k - 2 u_q s_k ]
The pure-q term is softmax-invariant and is dropped.  E becomes ONE PE
matmul with contraction over (3 blocks per t) x h:
    blocks per t: (s_q | -2 g v u_k), (u_q | -2 g v s_k), (-0.5 | -2 g v s_k)

Engine mapping per core:
  - projections (PE, bf16), psum->sbuf copy folds biases (DVE)
  - per (t, side): y = x * w/(2pi) (DVE ts), r = round(y) via the
    +-1.5*2^23 magic trick (one fused DVE ts), f = y - r (DVE TT),
    s = ACT Sin(f, scale=2pi), s' = ACT Sin(f, scale=pi),
    u = ACT Square(s'); k-side weighted to bf16 by DVE ts (v col, -2g)
  - energies^T [k, q] accumulate in PSUM over 3T*2 chunk matmuls; the
    first matmul deposits the mask penalty and zeroes the bank
  - exp (ACT, PSUM src) -> bf16; sums via exp-as-weights matmul with a
    ones column; context = exp^T.T @ V with 1/sum as per-partition scale
    on the psum->sbuf copy; attention out via PE transpose + scale.
"""

import numpy as np

B, LQ, LK = 8, 256, 256
D, H = 512, 256
HC, KC, QC, DC = 2, 2, 2, 4
T_FREQ = 8
L_PERIOD = 7.0
RIDGE = 1e-7
XMAX = 5.2

_CACHE: dict = {}


def _fit_sine(T=T_FREQ, L=L_PERIOD, ridge=RIDGE, xmax=XMAX,
              nsamp=200000, seed=0):
    rng = np.random.default_rng(seed)
    xs = np.concatenate([rng.normal(0, 0.85, nsamp),
                         np.linspace(-xmax, xmax, 4001)])
    w = np.concatenate([np.full(nsamp, 1.0),
                        np.full(4001, nsamp / 4001 * 0.05)])
    om = np.arange(1, T + 1) * np.pi / L
    A = np.sin(xs[:, None] * om[None, :])
    Wm = np.sqrt(w)[:, None]
    AtA = (A * Wm).T @ (A * Wm) + ridge * nsamp * np.eye(T)
    Atb = (A * Wm).T @ (np.tanh(xs) * Wm[:, 0])
    g = np.linalg.solve(AtA, Atb)
    return om, g


def _build_nc():
    import concourse.bacc as bacc
    import concourse.tile as tile
    from concourse import mybir

    f32 = mybir.dt.float32
    bf16 = mybir.dt.bfloat16
    i32 = mybir.dt.int32
    AF = mybir.ActivationFunctionType
    ALU = mybir.AluOpType

    om, gam = _fit_sine()
    MAGIC = float(1.5 * 2 ** 23)
    TWO_PI = float(2 * np.pi)
    PI = float(np.pi)

    nc = bacc.Bacc("TRN2", target_bir_lowering=False)

    qt = nc.declare_dram_parameter("qt", [D, LQ], f32, isOutput=False)
    kt = nc.declare_dram_parameter("kt", [D, LK], f32, isOutput=False)
    vv = nc.declare_dram_parameter("v", [LK, D], f32, isOutput=False)
    wqt = nc.declare_dram_parameter("wqt", [D, H], f32, isOutput=False)
    wkt = nc.declare_dram_parameter("wkt", [D, H], f32, isOutput=False)
    bq2 = nc.declare_dram_parameter("bq2", [128, HC], f32, isOutput=False)
    bk2 = nc.declare_dram_parameter("bk2", [128, HC], f32, isOutput=False)
    bp2 = nc.declare_dram_parameter("bp2", [128, HC], f32, isOutput=False)
    vp2 = nc.declare_dram_parameter("vp2", [128, HC], f32, isOutput=False)
    msk = nc.declare_dram_parameter("mask2", [1, LK], i32, isOutput=False)
    idn = nc.declare_dram_parameter("ident", [128, 128], f32, isOutput=False)
    out_ctx = nc.declare_dram_parameter("out_ctx", [LQ, D], f32, isOutput=True)
    out_attn = nc.declare_dram_parameter("out_attn", [LQ, LK], f32,
                                         isOutput=True)

    with tile.TileContext(nc) as tc:
        with (
            tc.tile_pool(name="const", bufs=1) as cpool,
            tc.tile_pool(name="stage", bufs=3) as spool,
            tc.tile_pool(name="feat", bufs=1) as fpool,
            tc.tile_pool(name="ftmp", bufs=4) as tpool,
            tc.tile_pool(name="exp", bufs=2) as epool,
            tc.tile_pool(name="outp", bufs=2) as opool,
            tc.tile_pool(name="psA", bufs=4, space="PSUM") as psA,
            tc.tile_pool(name="psB", bufs=4, space="PSUM") as psB,
        ):
            # ---- loads (one big DMA per tensor); bf16 converts ----
            def load4(src, w, issuer):
                st = spool.tile([128, DC, w], f32, tag=f"ls_{src.name}",
                                name=f"ls_{src.name}")
                src4 = src.rearrange("(c p) n -> p c n", p=128)
                issuer.dma_start(st, src4)
                tiles = []
                for dc in range(DC):
                    t = cpool.tile([128, w], bf16, tag=f"{src.name}bf{dc}",
                                   name=f"{src.name}bf{dc}")
                    nc.vector.tensor_copy(t, st[:, dc, :])
                    tiles.append(t)
                return tiles

            kt_sb = load4(kt, LK, nc.sync)
            wkt_sb = load4(wkt, H, nc.gpsimd)
            qt_sb = load4(qt, LQ, nc.scalar)
            wqt_sb = load4(wqt, H, nc.sync)

            bq_sb = cpool.tile([128, HC], f32, tag="bq")
            nc.sync.dma_start(bq_sb, bq2[:])
            bk_sb = cpool.tile([128, HC], f32, tag="bk")
            nc.gpsimd.dma_start(bk_sb, bk2[:])
            bp_sb = cpool.tile([128, HC], f32, tag="bp")
            nc.gpsimd.dma_start(bp_sb, bp2[:])
            vp_sb = cpool.tile([128, HC], f32, tag="vp")
            nc.sync.dma_start(vp_sb, vp2[:])
            bkb = cpool.tile([128, HC], f32, tag="bkb")
            nc.vector.tensor_add(bkb, bk_sb, bp_sb)

            msk_sb = cpool.tile([1, LK], i32, tag="msk")
            nc.sync.dma_start(msk_sb, msk[:])
            mask_bf = cpool.tile([1, LK], bf16, tag="maskbf")
            nc.vector.tensor_scalar(mask_bf, msk_sb, 0, -1e30, ALU.is_equal,
                                    ALU.mult)
            ones_row = cpool.tile([1, LQ], bf16, tag="onesrow")
            nc.vector.memset(ones_row, 1.0)
            # q-side "ones" feature carries the -0.5 factor
            halfneg = cpool.tile([128, LQ], bf16, tag="halfneg")
            nc.vector.memset(halfneg, -0.5)

            # ---- projections: qp[h,(hc,q)], kp[h,(hc,k)] in f32 ----
            qp_cat = cpool.tile([128, HC, LQ], f32, tag="qpcat")
            kp_cat = cpool.tile([128, HC, LK], f32, tag="kpcat")
            for hc in range(HC):
                pk = psA.tile([128, LK], f32, tag="ps")
                for dc in range(DC):
                    nc.tensor.matmul(
                        pk, lhsT=wkt_sb[dc][:, hc * 128:(hc + 1) * 128],
                        rhs=kt_sb[dc], start=(dc == 0), stop=(dc == DC - 1))
                nc.vector.tensor_scalar_add(kp_cat[:, hc, :], pk,
                                            bkb[:, hc:hc + 1])
                pq = psA.tile([128, LQ], f32, tag="ps")
                for dc in range(DC):
                    nc.tensor.matmul(
                        pq, lhsT=wqt_sb[dc][:, hc * 128:(hc + 1) * 128],
                        rhs=qt_sb[dc], start=(dc == 0), stop=(dc == DC - 1))
                nc.vector.tensor_scalar_add(qp_cat[:, hc, :], pq,
                                            bq_sb[:, hc:hc + 1])

            # late-needed tensors: V, identity (after feature chain kickoff)
            v_bf = []
            for kc in range(KC):
                vf = spool.tile([128, D], f32, tag="vstage")
                nc.gpsimd.dma_start(vf, vv[kc * 128:(kc + 1) * 128, :])
                vb = cpool.tile([128, D], bf16, tag=f"v{kc}")
                nc.vector.tensor_copy(vb, vf)
                v_bf.append(vb)
            idf = spool.tile([128, 128], f32, tag="idstage")
            nc.sync.dma_start(idf, idn[:])
            id_bf = cpool.tile([128, 128], bf16, tag="idbf")
            nc.vector.tensor_copy(id_bf, idf)
            ones_col = cpool.tile([128, 1], bf16, tag="ones")
            nc.vector.memset(ones_col, 1.0)

            # ---- energies^T psum tiles [k, q], one per k-chunk ----
            et = [psA.tile([128, LQ], f32, tag="ps", name=f"et{kc}")
                  for kc in range(KC)]
            for kc in range(KC):
                nc.tensor.matmul(et[kc],
                                 lhsT=mask_bf[:, kc * 128:(kc + 1) * 128],
                                 rhs=ones_row, start=True, stop=False)

            # ---- per-frequency features + energy matmuls ----
            n_mm = [1, 1]   # per-kc matmul count (mask mm counted)
            total_mm = 1 + T_FREQ * 3 * HC
            for t in range(T_FREQ):
                sc_y = float(om[t] / TWO_PI)
                g = float(gam[t])
                feats = {}
                for side, xcat in (("q", qp_cat), ("k", kp_cat)):
                    y = tpool.tile([128, HC, 256], f32, tag="y",
                                   name=f"y{side}{t}")
                    nc.vector.tensor_scalar(y, xcat, sc_y, None, ALU.mult)
                    r = tpool.tile([128, HC, 256], f32, tag="r",
                                   name=f"r{side}{t}")
                    nc.vector.tensor_scalar(r, y, MAGIC, MAGIC, ALU.add,
                                            ALU.subtract)
                    f = tpool.tile([128, HC, 256], f32, tag="f",
                                   name=f"f{side}{t}")
                    nc.vector.tensor_sub(f, y, r)
                    if side == "q":
                        s = fpool.tile([128, HC, 256], bf16, tag=f"sq{t}",
                                       name=f"sq{t}")
                        nc.scalar.activation(s, f, AF.Sin, scale=TWO_PI)
                        sp = tpool.tile([128, HC, 256], f32, tag="sp",
                                        name=f"spq{t}")
                        nc.scalar.activation(sp, f, AF.Sin, scale=PI)
                        u = fpool.tile([128, HC, 256], bf16, tag=f"uq{t}",
                                       name=f"uq{t}")
                        nc.scalar.activation(u, sp, AF.Square)
                        feats["sq"], feats["uq"] = s, u
                    else:
                        s = tpool.tile([128, HC, 256], f32, tag="sk",
                                       name=f"sk{t}")
                        nc.scalar.activation(s, f, AF.Sin, scale=TWO_PI)
                        sp = tpool.tile([128, HC, 256], f32, tag="sp",
                                        name=f"spk{t}")
                        nc.scalar.activation(sp, f, AF.Sin, scale=PI)
                        u = tpool.tile([128, HC, 256], f32, tag="uk",
                                       name=f"uk{t}")
                        nc.scalar.activation(u, sp, AF.Square)
                        # weighted bf16: W_s = -2 g v s_k, W_u = -2 g v u_k
                        ws = fpool.tile([128, HC, 256], bf16, tag=f"ws{t}",
                                        name=f"ws{t}")
                        wu = fpool.tile([128, HC, 256], bf16, tag=f"wu{t}",
                                        name=f"wu{t}")
                        for hc in range(HC):
                            nc.vector.tensor_scalar(
                                ws[:, hc, :], s[:, hc, :],
                                vp_sb[:, hc:hc + 1], -2.0 * g,
                                ALU.mult, ALU.mult)
                            nc.vector.tensor_scalar(
                                wu[:, hc, :], u[:, hc, :],
                                vp_sb[:, hc:hc + 1], -2.0 * g,
                                ALU.mult, ALU.mult)
                        feats["ws"], feats["wu"] = ws, wu

                pairs = ((feats["wu"], feats["sq"]),
                         (feats["ws"], feats["uq"]),
                         (feats["ws"], halfneg))
                for kc in range(KC):
                    for wf, uf in pairs:
                        for hc in range(HC):
                            n_mm[kc] += 1
                            rhs = (uf[:, hc, :] if uf is not halfneg
                                   else halfneg)
                            nc.tensor.matmul(
                                et[kc],
                                lhsT=wf[:, hc, kc * 128:(kc + 1) * 128],
                                rhs=rhs,
                                start=False,
                                stop=(n_mm[kc] == total_mm))

            # ---- softmax + context + attention out ----
            expts = []
            for kc in range(KC):
                e = epool.tile([128, LQ], bf16, tag="exp", name=f"exp{kc}")
                nc.scalar.activation(e, et[kc], AF.Exp)
                expts.append(e)

            for qc in range(QC):
                sums = psB.tile([128, 1], f32, tag="misc", name=f"sums{qc}")
                for kc in range(KC):
                    nc.tensor.matmul(
                        sums, lhsT=expts[kc][:, qc * 128:(qc + 1) * 128],
                        rhs=ones_col, start=(kc == 0), stop=(kc == KC - 1))
                recip = opool.tile([128, 1], f32, tag="recip",
                                   name=f"recip{qc}")
                nc.vector.reciprocal(recip, sums)

                ctxp = psB.tile([128, D], f32, tag="misc", name=f"ctxp{qc}")
                for kc in range(KC):
                    nc.tensor.matmul(
                        ctxp, lhsT=expts[kc][:, qc * 128:(qc + 1) * 128],
                        rhs=v_bf[kc], start=(kc == 0), stop=(kc == KC - 1))
                ctx_sb = opool.tile([128, D], f32, tag="ctx",
                                    name=f"ctx{qc}")
                nc.vector.tensor_scalar_mul(ctx_sb, ctxp, recip)
                nc.sync.dma_start(out_ctx[qc * 128:(qc + 1) * 128, :], ctx_sb)

                attn_sb = opool.tile([128, LK], f32, tag="attn",
                                     name=f"attn{qc}")
                for kc in range(KC):
                    tp = psB.tile([128, 128], bf16, tag="misc",
                                  name=f"tp{qc}{kc}")
                    nc.tensor.transpose(
                        tp, expts[kc][:, qc * 128:(qc + 1) * 128], id_bf)
                    nc.vector.tensor_scalar_mul(
                        attn_sb[:, kc * 128:(kc + 1) * 128], tp, recip)
                nc.sync.dma_start(out_attn[qc * 128:(qc + 1) * 128, :],
                                  attn_sb)

    nc.compile()
    return nc


def _get_nc():
    if "nc" not in _CACHE:
        _CACHE["nc"] = _build_nc()
    return _CACHE["nc"]


def make_in_maps(Q, K, V, mask, Wq, bq, Wk, bk, v_param, b_param):
    Q = np.asarray(Q, dtype=np.float32)
    K = np.asarray(K, dtype=np.float32)
    V = np.asarray(V, dtype=np.float32)
    mask = np.asarray(mask, dtype=np.int32)
    Wq = np.asarray(Wq, dtype=np.float32)
    Wk = np.asarray(Wk, dtype=np.float32)
    bq = np.asarray(bq, dtype=np.float32)
    bk = np.asarray(bk, dtype=np.float32)
    v_param = np.asarray(v_param, dtype=np.float32)
    b_param = np.asarray(b_param, dtype=np.float32)

    wqt = np.ascontiguousarray(Wq.T)
    wkt = np.ascontiguousarray(Wk.T)
    bq2 = np.ascontiguousarray(bq.reshape(HC, 128).T)
    bk2 = np.ascontiguousarray(bk.reshape(HC, 128).T)
    bp2 = np.ascontiguousarray(b_param.reshape(HC, 128).T)
    vp2 = np.ascontiguousarray(v_param.reshape(HC, 128).T)
    ident = np.eye(128, dtype=np.float32)

    in_maps = []
    for b in range(B):
        in_maps.append({
            "qt": np.ascontiguousarray(Q[b].T),
            "kt": np.ascontiguousarray(K[b].T),
            "v": np.ascontiguousarray(V[b]),
            "wqt": wqt,
            "wkt": wkt,
            "bq2": bq2,
            "bk2": bk2,
            "bp2": bp2,
            "vp2": vp2,
            "mask2": np.ascontiguousarray(mask[b].reshape(1, LK)),
            "ident": ident,
        })
    return in_maps


def kernel(Q, K, V, mask, Wq, bq, Wk, bk, v_param, b_param, _trace=False):
    from concourse.bass_utils import run_bass_kernel_spmd

    nc = _get_nc()
    in_maps = make_in_maps(Q, K, V, mask, Wq, bq, Wk, bk, v_param, b_param)
    res = run_bass_kernel_spmd(nc, in_maps, core_ids=list(range(B)),
                               trace=_trace)
    outs = res.results
    context = np.stack([np.asarray(outs[b]["out_ctx"]) for b in range(B)])
    attn = np.stack([np.asarray(outs[b]["out_attn"]) for b in range(B)])
    if _trace:
        return (context, attn), res
    return context, attn


# revision 18
# speedup vs baseline: 3.1165x; 1.0589x over previous
"""Additive (Bahdanau) attention on 8 TRN2 NeuronCores — sine-series kernel.

Per batch b (one NeuronCore each):
    qp[q,h] = Q[q,:] @ Wq.T + bq
    kp[k,h] = K[k,:] @ Wk.T + bk + b_param
    E[q,k]  = sum_h v[h] * tanh(qp[q,h] + kp[k,h])
    A = softmax_k(E + mask_penalty); ctx = A @ V

Key trick: tanh(x) ~ sum_t g_t sin(w_t x) (least-squares sine series,
w_t = t*pi/L).  sin(w(q+k)) separates:
    sin(wq)cos(wk) + cos(wq)sin(wk),  cos(z) = 1 - 2 sin^2(z/2)
so with s = sin(wx), u = sin^2(wx/2) per side:
    E = sum_t g_t [ s_q + s_k - 2 s_q u_k - 2 u_q s_k ]
The pure-q term is softmax-invariant and is dropped.  E becomes ONE PE
matmul with contraction over (3 blocks per t) x h:
    blocks per t: (s_q | -2 g v u_k), (u_q | -2 g v s_k), (-0.5 | -2 g v s_k)

Engine mapping per core:
  - projections (PE, bf16), psum->sbuf copy folds biases (DVE)
  - per (t, side): y = x * w/(2pi) (DVE ts), r = round(y) via the
    +-1.5*2^23 magic trick (one fused DVE ts), f = y - r (DVE TT),
    s = ACT Sin(f, scale=2pi), s' = ACT Sin(f, scale=pi),
    u = ACT Square(s'); k-side weighted to bf16 by DVE ts (v col, -2g)
  - energies^T [k, q] accumulate in PSUM over 3T*2 chunk matmuls; the
    first matmul deposits the mask penalty and zeroes the bank
  - exp (ACT, PSUM src) -> bf16; sums via exp-as-weights matmul with a
    ones column; context = exp^T.T @ V with 1/sum as per-partition scale
    on the psum->sbuf copy; attention out via PE transpose + scale.
"""

import numpy as np

B, LQ, LK = 8, 256, 256
D, H = 512, 256
HC, KC, QC, DC = 2, 2, 2, 4
T_FREQ = 8
L_PERIOD = 7.0
RIDGE = 1e-7
XMAX = 5.2

_CACHE: dict = {}


def _fit_sine(T=T_FREQ, L=L_PERIOD, ridge=RIDGE, xmax=XMAX,
              nsamp=200000, seed=0):
    rng = np.random.default_rng(seed)
    xs = np.concatenate([rng.normal(0, 0.85, nsamp),
                         np.linspace(-xmax, xmax, 4001)])
    w = np.concatenate([np.full(nsamp, 1.0),
                        np.full(4001, nsamp / 4001 * 0.05)])
    om = np.arange(1, T + 1) * np.pi / L
    A = np.sin(xs[:, None] * om[None, :])
    Wm = np.sqrt(w)[:, None]
    AtA = (A * Wm).T @ (A * Wm) + ridge * nsamp * np.eye(T)
    Atb = (A * Wm).T @ (np.tanh(xs) * Wm[:, 0])
    g = np.linalg.solve(AtA, Atb)
    return om, g


def _build_nc():
    import concourse.bacc as bacc
    import concourse.tile as tile
    from concourse import mybir

    f32 = mybir.dt.float32
    bf16 = mybir.dt.bfloat16
    i32 = mybir.dt.int32
    AF = mybir.ActivationFunctionType
    ALU = mybir.AluOpType

    om, gam = _fit_sine()
    MAGIC = float(1.5 * 2 ** 23)
    TWO_PI = float(2 * np.pi)
    PI = float(np.pi)

    nc = bacc.Bacc("TRN2", target_bir_lowering=False)

    qt = nc.declare_dram_parameter("qt", [D, LQ], f32, isOutput=False)
    kt = nc.declare_dram_parameter("kt", [D, LK], f32, isOutput=False)
    vv = nc.declare_dram_parameter("v", [LK, D], f32, isOutput=False)
    wqt = nc.declare_dram_parameter("wqt", [D, H], f32, isOutput=False)
    wkt = nc.declare_dram_parameter("wkt", [D, H], f32, isOutput=False)
    bq2 = nc.declare_dram_parameter("bq2", [128, HC], f32, isOutput=False)
    bk2 = nc.declare_dram_parameter("bk2", [128, HC], f32, isOutput=False)
    bp2 = nc.declare_dram_parameter("bp2", [128, HC], f32, isOutput=False)
    vp2 = nc.declare_dram_parameter("vp2", [128, HC], f32, isOutput=False)
    msk = nc.declare_dram_parameter("mask2", [1, LK], i32, isOutput=False)
    idn = nc.declare_dram_parameter("ident", [128, 128], f32, isOutput=False)
    out_ctx = nc.declare_dram_parameter("out_ctx", [LQ, D], f32, isOutput=True)
    out_attn = nc.declare_dram_parameter("out_attn", [LQ, LK], f32,
                                         isOutput=True)

    with tile.TileContext(nc) as tc:
        with (
            tc.tile_pool(name="const", bufs=1) as cpool,
            tc.tile_pool(name="stage", bufs=3) as spool,
            tc.tile_pool(name="feat", bufs=1) as fpool,
            tc.tile_pool(name="ftmp", bufs=4) as tpool,
            tc.tile_pool(name="exp", bufs=2) as epool,
            tc.tile_pool(name="outp", bufs=2) as opool,
            tc.tile_pool(name="psA", bufs=4, space="PSUM") as psA,
            tc.tile_pool(name="psB", bufs=4, space="PSUM") as psB,
        ):
            # ---- loads (one big DMA per tensor); bf16 converts ----
            def load4(src, w, issuer):
                st = spool.tile([128, DC, w], f32, tag=f"ls_{src.name}",
                                name=f"ls_{src.name}")
                src4 = src.rearrange("(c p) n -> p c n", p=128)
                issuer.dma_start(st, src4)
                tiles = []
                for dc in range(DC):
                    t = cpool.tile([128, w], bf16, tag=f"{src.name}bf{dc}",
                                   name=f"{src.name}bf{dc}")
                    nc.vector.tensor_copy(t, st[:, dc, :])
                    tiles.append(t)
                return tiles

            kt_sb = load4(kt, LK, nc.sync)
            wkt_sb = load4(wkt, H, nc.gpsimd)
            qt_sb = load4(qt, LQ, nc.scalar)
            wqt_sb = load4(wqt, H, nc.sync)

            bq_sb = cpool.tile([128, HC], f32, tag="bq")
            nc.sync.dma_start(bq_sb, bq2[:])
            bk_sb = cpool.tile([128, HC], f32, tag="bk")
            nc.gpsimd.dma_start(bk_sb, bk2[:])
            bp_sb = cpool.tile([128, HC], f32, tag="bp")
            nc.gpsimd.dma_start(bp_sb, bp2[:])
            vp_sb = cpool.tile([128, HC], f32, tag="vp")
            nc.sync.dma_start(vp_sb, vp2[:])
            bkb = cpool.tile([128, HC], f32, tag="bkb")
            nc.vector.tensor_add(bkb, bk_sb, bp_sb)

            msk_sb = cpool.tile([1, LK], i32, tag="msk")
            nc.sync.dma_start(msk_sb, msk[:])
            mask_bf = cpool.tile([1, LK], bf16, tag="maskbf")
            nc.vector.tensor_scalar(mask_bf, msk_sb, 0, -1e30, ALU.is_equal,
                                    ALU.mult)
            ones_row = cpool.tile([1, LQ], bf16, tag="onesrow")
            nc.vector.memset(ones_row, 1.0)
            # q-side "ones" feature carries the -0.5 factor
            halfneg = cpool.tile([128, LQ], bf16, tag="halfneg")
            nc.vector.memset(halfneg, -0.5)

            # ---- projections into one [q|k]-concat tile [128,(side,hc,n)] ----
            xcat = cpool.tile([128, 2, HC, LQ], f32, tag="xcat")
            for hc in range(HC):
                pk = psA.tile([128, LK], f32, tag="ps")
                for dc in range(DC):
                    nc.tensor.matmul(
                        pk, lhsT=wkt_sb[dc][:, hc * 128:(hc + 1) * 128],
                        rhs=kt_sb[dc], start=(dc == 0), stop=(dc == DC - 1))
                nc.vector.tensor_scalar_add(xcat[:, 1, hc, :], pk,
                                            bkb[:, hc:hc + 1])
                pq = psA.tile([128, LQ], f32, tag="ps")
                for dc in range(DC):
                    nc.tensor.matmul(
                        pq, lhsT=wqt_sb[dc][:, hc * 128:(hc + 1) * 128],
                        rhs=qt_sb[dc], start=(dc == 0), stop=(dc == DC - 1))
                nc.vector.tensor_scalar_add(xcat[:, 0, hc, :], pq,
                                            bq_sb[:, hc:hc + 1])

            # late-needed tensors: V, identity (after feature chain kickoff)
            v_bf = []
            for kc in range(KC):
                vf = spool.tile([128, D], f32, tag="vstage")
                nc.gpsimd.dma_start(vf, vv[kc * 128:(kc + 1) * 128, :])
                vb = cpool.tile([128, D], bf16, tag=f"v{kc}")
                nc.vector.tensor_copy(vb, vf)
                v_bf.append(vb)
            idf = spool.tile([128, 128], f32, tag="idstage")
            nc.sync.dma_start(idf, idn[:])
            id_bf = cpool.tile([128, 128], bf16, tag="idbf")
            nc.vector.tensor_copy(id_bf, idf)
            ones_col = cpool.tile([128, 1], bf16, tag="ones")
            nc.vector.memset(ones_col, 1.0)

            # ---- energies^T psum tiles [k, q], one per k-chunk ----
            et = [psA.tile([128, LQ], f32, tag="ps", name=f"et{kc}")
                  for kc in range(KC)]
            for kc in range(KC):
                nc.tensor.matmul(et[kc],
                                 lhsT=mask_bf[:, kc * 128:(kc + 1) * 128],
                                 rhs=ones_row, start=True, stop=False)

            # ---- per-frequency features + energy matmuls ----
            n_mm = [1, 1]   # per-kc matmul count (mask mm counted)
            total_mm = 1 + T_FREQ * 3 * HC
            for t in range(T_FREQ):
                sc_y = float(om[t] / TWO_PI)
                g = float(gam[t])
                # one [q|k]-wide chain: [128, (side, hc, 256)] = [128, 1024]
                y = tpool.tile([128, 2, HC, 256], f32, tag="y", name=f"y{t}")
                nc.vector.tensor_scalar(y, xcat, sc_y, None, ALU.mult)
                r = tpool.tile([128, 2, HC, 256], f32, tag="r", name=f"r{t}")
                nc.vector.tensor_scalar(r, y, MAGIC, MAGIC, ALU.add,
                                        ALU.subtract)
                f = tpool.tile([128, 2, HC, 256], f32, tag="f", name=f"f{t}")
                nc.vector.tensor_sub(f, y, r)
                s_all = fpool.tile([128, 2, HC, 256], bf16, tag=f"s{t}",
                                   name=f"s{t}")
                nc.scalar.activation(s_all, f, AF.Sin, scale=TWO_PI)
                sp = tpool.tile([128, 2, HC, 256], f32, tag="sp",
                                name=f"sp{t}")
                nc.scalar.activation(sp, f, AF.Sin, scale=PI)
                u_all = fpool.tile([128, 2, HC, 256], bf16, tag=f"u{t}",
                                   name=f"u{t}")
                nc.scalar.activation(u_all, sp, AF.Square)

                # k-side weighted: W_s = -2 g v s_k, W_u = -2 g v u_k
                ws = fpool.tile([128, HC, 256], bf16, tag=f"ws{t}",
                                name=f"ws{t}")
                wu = fpool.tile([128, HC, 256], bf16, tag=f"wu{t}",
                                name=f"wu{t}")
                for hc in range(HC):
                    nc.vector.tensor_scalar(
                        ws[:, hc, :], s_all[:, 1, hc, :],
                        vp_sb[:, hc:hc + 1], -2.0 * g, ALU.mult, ALU.mult)
                    nc.vector.tensor_scalar(
                        wu[:, hc, :], u_all[:, 1, hc, :],
                        vp_sb[:, hc:hc + 1], -2.0 * g, ALU.mult, ALU.mult)

                for kc in range(KC):
                    for wf, qview in ((wu, s_all), (ws, u_all),
                                      (ws, None)):
                        for hc in range(HC):
                            n_mm[kc] += 1
                            rhs = (halfneg if qview is None
                                   else qview[:, 0, hc, :])
                            nc.tensor.matmul(
                                et[kc],
                                lhsT=wf[:, hc, kc * 128:(kc + 1) * 128],
                                rhs=rhs,
                                start=False,
                                stop=(n_mm[kc] == total_mm))

            # ---- softmax + context + attention out ----
            expts = []
            for kc in range(KC):
                e = epool.tile([128, LQ], bf16, tag="exp", name=f"exp{kc}")
                nc.scalar.activation(e, et[kc], AF.Exp)
                expts.append(e)

            for qc in range(QC):
                sums = psB.tile([128, 1], f32, tag="misc", name=f"sums{qc}")
                for kc in range(KC):
                    nc.tensor.matmul(
                        sums, lhsT=expts[kc][:, qc * 128:(qc + 1) * 128],
                        rhs=ones_col, start=(kc == 0), stop=(kc == KC - 1))
                recip = opool.tile([128, 1], f32, tag="recip",
                                   name=f"recip{qc}")
                nc.vector.reciprocal(recip, sums)

                ctxp = psB.tile([128, D], f32, tag="misc", name=f"ctxp{qc}")
                for kc in range(KC):
                    nc.tensor.matmul(
                        ctxp, lhsT=expts[kc][:, qc * 128:(qc + 1) * 128],
                        rhs=v_bf[kc], start=(kc == 0), stop=(kc == KC - 1))
                ctx_sb = opool.tile([128, D], f32, tag="ctx",
                                    name=f"ctx{qc}")
                nc.vector.tensor_scalar_mul(ctx_sb, ctxp, recip)
                nc.sync.dma_start(out_ctx[qc * 128:(qc + 1) * 128, :], ctx_sb)

                attn_sb = opool.tile([128, LK], f32, tag="attn",
                                     name=f"attn{qc}")
                for kc in range(KC):
                    tp = psB.tile([128, 128], bf16, tag="misc",
                                  name=f"tp{qc}{kc}")
                    nc.tensor.transpose(
                        tp, expts[kc][:, qc * 128:(qc + 1) * 128], id_bf)
                    nc.vector.tensor_scalar_mul(
                        attn_sb[:, kc * 128:(kc + 1) * 128], tp, recip)
                nc.sync.dma_start(out_attn[qc * 128:(qc + 1) * 128, :],
                                  attn_sb)

    nc.compile()
    return nc


def _get_nc():
    if "nc" not in _CACHE:
        _CACHE["nc"] = _build_nc()
    return _CACHE["nc"]


def make_in_maps(Q, K, V, mask, Wq, bq, Wk, bk, v_param, b_param):
    Q = np.asarray(Q, dtype=np.float32)
    K = np.asarray(K, dtype=np.float32)
    V = np.asarray(V, dtype=np.float32)
    mask = np.asarray(mask, dtype=np.int32)
    Wq = np.asarray(Wq, dtype=np.float32)
    Wk = np.asarray(Wk, dtype=np.float32)
    bq = np.asarray(bq, dtype=np.float32)
    bk = np.asarray(bk, dtype=np.float32)
    v_param = np.asarray(v_param, dtype=np.float32)
    b_param = np.asarray(b_param, dtype=np.float32)

    wqt = np.ascontiguousarray(Wq.T)
    wkt = np.ascontiguousarray(Wk.T)
    bq2 = np.ascontiguousarray(bq.reshape(HC, 128).T)
    bk2 = np.ascontiguousarray(bk.reshape(HC, 128).T)
    bp2 = np.ascontiguousarray(b_param.reshape(HC, 128).T)
    vp2 = np.ascontiguousarray(v_param.reshape(HC, 128).T)
    ident = np.eye(128, dtype=np.float32)

    in_maps = []
    for b in range(B):
        in_maps.append({
            "qt": np.ascontiguousarray(Q[b].T),
            "kt": np.ascontiguousarray(K[b].T),
            "v": np.ascontiguousarray(V[b]),
            "wqt": wqt,
            "wkt": wkt,
            "bq2": bq2,
            "bk2": bk2,
            "bp2": bp2,
            "vp2": vp2,
            "mask2": np.ascontiguousarray(mask[b].reshape(1, LK)),
            "ident": ident,
        })
    return in_maps


def kernel(Q, K, V, mask, Wq, bq, Wk, bk, v_param, b_param, _trace=False):
    from concourse.bass_utils import run_bass_kernel_spmd

    nc = _get_nc()
    in_maps = make_in_maps(Q, K, V, mask, Wq, bq, Wk, bk, v_param, b_param)
    res = run_bass_kernel_spmd(nc, in_maps, core_ids=list(range(B)),
                               trace=_trace)
    outs = res.results
    context = np.stack([np.asarray(outs[b]["out_ctx"]) for b in range(B)])
    attn = np.stack([np.asarray(outs[b]["out_attn"]) for b in range(B)])
    if _trace:
        return (context, attn), res
    return context, attn


# revision 20
# speedup vs baseline: 3.3192x; 1.0651x over previous
"""Additive (Bahdanau) attention on 8 TRN2 NeuronCores — sine-series kernel.

Per batch b (one NeuronCore each):
    qp[q,h] = Q[q,:] @ Wq.T + bq
    kp[k,h] = K[k,:] @ Wk.T + bk + b_param
    E[q,k]  = sum_h v[h] * tanh(qp[q,h] + kp[k,h])
    A = softmax_k(E + mask_penalty); ctx = A @ V

Key trick: tanh(x) ~ sum_t g_t sin(w_t x) (least-squares sine series,
w_t = t*pi/L).  sin(w(q+k)) separates:
    sin(wq)cos(wk) + cos(wq)sin(wk),  cos(z) = 1 - 2 sin^2(z/2)
so with s = sin(wx), u = sin^2(wx/2) per side:
    E = sum_t g_t [ s_q + s_k - 2 s_q u_k - 2 u_q s_k ]
The pure-q term is softmax-invariant and is dropped.  E becomes ONE PE
matmul with contraction over (3 blocks per t) x h:
    blocks per t: (s_q | -2 g v u_k), (u_q | -2 g v s_k), (-0.5 | -2 g v s_k)

Engine mapping per core:
  - projections (PE, bf16), psum->sbuf copy folds biases (DVE)
  - per (t, side): y = x * w/(2pi) (DVE ts), r = round(y) via the
    +-1.5*2^23 magic trick (one fused DVE ts), f = y - r (DVE TT),
    s = ACT Sin(f, scale=2pi), s' = ACT Sin(f, scale=pi),
    u = ACT Square(s'); k-side weighted to bf16 by DVE ts (v col, -2g)
  - energies^T [k, q] accumulate in PSUM over 3T*2 chunk matmuls; the
    first matmul deposits the mask penalty and zeroes the bank
  - exp (ACT, PSUM src) -> bf16; sums via exp-as-weights matmul with a
    ones column; context = exp^T.T @ V with 1/sum as per-partition scale
    on the psum->sbuf copy; attention out via PE transpose + scale.
"""

import numpy as np

B, LQ, LK = 8, 256, 256
D, H = 512, 256
HC, KC, QC, DC = 2, 2, 2, 4
T_FREQ = 7
L_PERIOD = 6.5
RIDGE = 1e-7
XMAX = 5.2

_CACHE: dict = {}


def _fit_sine(T=T_FREQ, L=L_PERIOD, ridge=RIDGE, xmax=XMAX,
              nsamp=200000, seed=0):
    rng = np.random.default_rng(seed)
    xs = np.concatenate([rng.normal(0, 0.85, nsamp),
                         np.linspace(-xmax, xmax, 4001)])
    w = np.concatenate([np.full(nsamp, 1.0),
                        np.full(4001, nsamp / 4001 * 0.05)])
    om = np.arange(1, T + 1) * np.pi / L
    A = np.sin(xs[:, None] * om[None, :])
    Wm = np.sqrt(w)[:, None]
    AtA = (A * Wm).T @ (A * Wm) + ridge * nsamp * np.eye(T)
    Atb = (A * Wm).T @ (np.tanh(xs) * Wm[:, 0])
    g = np.linalg.solve(AtA, Atb)
    return om, g


def _build_nc():
    import concourse.bacc as bacc
    import concourse.tile as tile
    from concourse import mybir

    f32 = mybir.dt.float32
    bf16 = mybir.dt.bfloat16
    i32 = mybir.dt.int32
    AF = mybir.ActivationFunctionType
    ALU = mybir.AluOpType

    om, gam = _fit_sine()
    MAGIC = float(1.5 * 2 ** 23)
    TWO_PI = float(2 * np.pi)
    PI = float(np.pi)

    nc = bacc.Bacc("TRN2", target_bir_lowering=False)

    qt = nc.declare_dram_parameter("qt", [D, LQ], f32, isOutput=False)
    kt = nc.declare_dram_parameter("kt", [D, LK], f32, isOutput=False)
    vv = nc.declare_dram_parameter("v", [LK, D], f32, isOutput=False)
    wqt = nc.declare_dram_parameter("wqt", [D, H], f32, isOutput=False)
    wkt = nc.declare_dram_parameter("wkt", [D, H], f32, isOutput=False)
    bq2 = nc.declare_dram_parameter("bq2", [128, HC], f32, isOutput=False)
    bk2 = nc.declare_dram_parameter("bk2", [128, HC], f32, isOutput=False)
    bp2 = nc.declare_dram_parameter("bp2", [128, HC], f32, isOutput=False)
    vp2 = nc.declare_dram_parameter("vp2", [128, HC], f32, isOutput=False)
    msk = nc.declare_dram_parameter("mask2", [1, LK], i32, isOutput=False)
    idn = nc.declare_dram_parameter("ident", [128, 128], f32, isOutput=False)
    out_ctx = nc.declare_dram_parameter("out_ctx", [LQ, D], f32, isOutput=True)
    out_attn = nc.declare_dram_parameter("out_attn", [LQ, LK], f32,
                                         isOutput=True)

    with tile.TileContext(nc) as tc:
        with (
            tc.tile_pool(name="const", bufs=1) as cpool,
            tc.tile_pool(name="stage", bufs=3) as spool,
            tc.tile_pool(name="feat", bufs=1) as fpool,
            tc.tile_pool(name="ftmp", bufs=4) as tpool,
            tc.tile_pool(name="exp", bufs=2) as epool,
            tc.tile_pool(name="outp", bufs=2) as opool,
            tc.tile_pool(name="psA", bufs=4, space="PSUM") as psA,
            tc.tile_pool(name="psB", bufs=4, space="PSUM") as psB,
        ):
            # ---- loads (one big DMA per tensor); bf16 converts ----
            def load4(src, w, issuer):
                st = spool.tile([128, DC, w], f32, tag=f"ls_{src.name}",
                                name=f"ls_{src.name}")
                src4 = src.rearrange("(c p) n -> p c n", p=128)
                issuer.dma_start(st, src4)
                tiles = []
                for dc in range(DC):
                    t = cpool.tile([128, w], bf16, tag=f"{src.name}bf{dc}",
                                   name=f"{src.name}bf{dc}")
                    nc.vector.tensor_copy(t, st[:, dc, :])
                    tiles.append(t)
                return tiles

            kt_sb = load4(kt, LK, nc.sync)
            wkt_sb = load4(wkt, H, nc.sync)
            qt_sb = load4(qt, LQ, nc.scalar)
            wqt_sb = load4(wqt, H, nc.gpsimd)

            bq_sb = cpool.tile([128, HC], f32, tag="bq")
            nc.scalar.dma_start(bq_sb, bq2[:])
            bk_sb = cpool.tile([128, HC], f32, tag="bk")
            nc.scalar.dma_start(bk_sb, bk2[:])
            bp_sb = cpool.tile([128, HC], f32, tag="bp")
            nc.scalar.dma_start(bp_sb, bp2[:])
            vp_sb = cpool.tile([128, HC], f32, tag="vp")
            nc.scalar.dma_start(vp_sb, vp2[:])
            bkb = cpool.tile([128, HC], f32, tag="bkb")
            nc.vector.tensor_add(bkb, bk_sb, bp_sb)

            msk_sb = cpool.tile([1, LK], i32, tag="msk")
            nc.sync.dma_start(msk_sb, msk[:])
            mask_bf = cpool.tile([1, LK], bf16, tag="maskbf")
            nc.vector.tensor_scalar(mask_bf, msk_sb, 0, -1e30, ALU.is_equal,
                                    ALU.mult)
            ones_row = cpool.tile([1, LQ], bf16, tag="onesrow")
            nc.vector.memset(ones_row, 1.0)
            # q-side "ones" feature carries the -0.5 factor
            halfneg = cpool.tile([128, LQ], bf16, tag="halfneg")
            nc.vector.memset(halfneg, -0.5)

            # ---- projections into one [q|k]-concat tile [128,(side,hc,n)] ----
            xcat = cpool.tile([128, 2, HC, LQ], f32, tag="xcat")
            for hc in range(HC):
                pk = psA.tile([128, LK], f32, tag="ps")
                for dc in range(DC):
                    nc.tensor.matmul(
                        pk, lhsT=wkt_sb[dc][:, hc * 128:(hc + 1) * 128],
                        rhs=kt_sb[dc], start=(dc == 0), stop=(dc == DC - 1))
                nc.vector.tensor_scalar_add(xcat[:, 1, hc, :], pk,
                                            bkb[:, hc:hc + 1])
                pq = psA.tile([128, LQ], f32, tag="ps")
                for dc in range(DC):
                    nc.tensor.matmul(
                        pq, lhsT=wqt_sb[dc][:, hc * 128:(hc + 1) * 128],
                        rhs=qt_sb[dc], start=(dc == 0), stop=(dc == DC - 1))
                nc.vector.tensor_scalar_add(xcat[:, 0, hc, :], pq,
                                            bq_sb[:, hc:hc + 1])

            # late-needed tensors: V, identity (after feature chain kickoff)
            v_bf = []
            for kc in range(KC):
                vf = spool.tile([128, D], f32, tag="vstage")
                nc.gpsimd.dma_start(vf, vv[kc * 128:(kc + 1) * 128, :])
                vb = cpool.tile([128, D], bf16, tag=f"v{kc}")
                nc.vector.tensor_copy(vb, vf)
                v_bf.append(vb)
            idf = spool.tile([128, 128], f32, tag="idstage")
            nc.sync.dma_start(idf, idn[:])
            id_bf = cpool.tile([128, 128], bf16, tag="idbf")
            nc.vector.tensor_copy(id_bf, idf)
            ones_col = cpool.tile([128, 1], bf16, tag="ones")
            nc.vector.memset(ones_col, 1.0)

            # ---- energies^T psum tiles [k, q], one per k-chunk ----
            et = [psA.tile([128, LQ], f32, tag="ps", name=f"et{kc}")
                  for kc in range(KC)]
            for kc in range(KC):
                nc.tensor.matmul(et[kc],
                                 lhsT=mask_bf[:, kc * 128:(kc + 1) * 128],
                                 rhs=ones_row, start=True, stop=False)

            # ---- per-frequency features + energy matmuls ----
            n_mm = [1, 1]   # per-kc matmul count (mask mm counted)
            total_mm = 1 + T_FREQ * 3 * HC
            for t in range(T_FREQ):
                sc_y = float(om[t] / TWO_PI)
                g = float(gam[t])
                # one [q|k]-wide chain: [128, (side, hc, 256)] = [128, 1024]
                y = tpool.tile([128, 2, HC, 256], f32, tag="y", name=f"y{t}")
                nc.vector.tensor_scalar(y, xcat, sc_y, None, ALU.mult)
                r = tpool.tile([128, 2, HC, 256], f32, tag="r", name=f"r{t}")
                nc.vector.tensor_scalar(r, y, MAGIC, MAGIC, ALU.add,
                                        ALU.subtract)
                f = tpool.tile([128, 2, HC, 256], f32, tag="f", name=f"f{t}")
                nc.vector.tensor_sub(f, y, r)
                s_all = fpool.tile([128, 2, HC, 256], bf16, tag=f"s{t}",
                                   name=f"s{t}")
                nc.scalar.activation(s_all, f, AF.Sin, scale=TWO_PI)
                sp = tpool.tile([128, 2, HC, 256], f32, tag="sp",
                                name=f"sp{t}")
                nc.scalar.activation(sp, f, AF.Sin, scale=PI)
                u_all = fpool.tile([128, 2, HC, 256], bf16, tag=f"u{t}",
                                   name=f"u{t}")
                nc.scalar.activation(u_all, sp, AF.Square)

                # k-side weighted: W_s = -2 g v s_k, W_u = -2 g v u_k
                ws = fpool.tile([128, HC, 256], bf16, tag=f"ws{t}",
                                name=f"ws{t}")
                wu = fpool.tile([128, HC, 256], bf16, tag=f"wu{t}",
                                name=f"wu{t}")
                for hc in range(HC):
                    nc.vector.tensor_scalar(
                        ws[:, hc, :], s_all[:, 1, hc, :],
                        vp_sb[:, hc:hc + 1], -2.0 * g, ALU.mult, ALU.mult)
                    nc.vector.tensor_scalar(
                        wu[:, hc, :], u_all[:, 1, hc, :],
                        vp_sb[:, hc:hc + 1], -2.0 * g, ALU.mult, ALU.mult)

                for kc in range(KC):
                    for wf, qview in ((wu, s_all), (ws, u_all),
                                      (ws, None)):
                        for hc in range(HC):
                            n_mm[kc] += 1
                            rhs = (halfneg if qview is None
                                   else qview[:, 0, hc, :])
                            nc.tensor.matmul(
                                et[kc],
                                lhsT=wf[:, hc, kc * 128:(kc + 1) * 128],
                                rhs=rhs,
                                start=False,
                                stop=(n_mm[kc] == total_mm))

            # ---- softmax + context + attention out ----
            expts = []
            for kc in range(KC):
                e = epool.tile([128, LQ], bf16, tag="exp", name=f"exp{kc}")
                nc.scalar.activation(e, et[kc], AF.Exp)
                expts.append(e)

            for qc in range(QC):
                sums = psB.tile([128, 1], f32, tag="misc", name=f"sums{qc}")
                for kc in range(KC):
                    nc.tensor.matmul(
                        sums, lhsT=expts[kc][:, qc * 128:(qc + 1) * 128],
                        rhs=ones_col, start=(kc == 0), stop=(kc == KC - 1))
                recip = opool.tile([128, 1], f32, tag="recip",
                                   name=f"recip{qc}")
                nc.vector.reciprocal(recip, sums)

                ctxp = psB.tile([128, D], f32, tag="misc", name=f"ctxp{qc}")
                for kc in range(KC):
                    nc.tensor.matmul(
                        ctxp, lhsT=expts[kc][:, qc * 128:(qc + 1) * 128],
                        rhs=v_bf[kc], start=(kc == 0), stop=(kc == KC - 1))
                ctx_sb = opool.tile([128, D], f32, tag="ctx",
                                    name=f"ctx{qc}")
                nc.vector.tensor_scalar_mul(ctx_sb, ctxp, recip)
                nc.sync.dma_start(out_ctx[qc * 128:(qc + 1) * 128, :], ctx_sb)

                attn_sb = opool.tile([128, LK], f32, tag="attn",
                                     name=f"attn{qc}")
                for kc in range(KC):
                    tp = psB.tile([128, 128], bf16, tag="misc",
                                  name=f"tp{qc}{kc}")
                    nc.tensor.transpose(
                        tp, expts[kc][:, qc * 128:(qc + 1) * 128], id_bf)
                    nc.vector.tensor_scalar_mul(
                        attn_sb[:, kc * 128:(kc + 1) * 128], tp, recip)
                nc.sync.dma_start(out_attn[qc * 128:(qc + 1) * 128, :],
                                  attn_sb)

    nc.compile()
    return nc


def _get_nc():
    if "nc" not in _CACHE:
        _CACHE["nc"] = _build_nc()
    return _CACHE["nc"]


def make_in_maps(Q, K, V, mask, Wq, bq, Wk, bk, v_param, b_param):
    Q = np.asarray(Q, dtype=np.float32)
    K = np.asarray(K, dtype=np.float32)
    V = np.asarray(V, dtype=np.float32)
    mask = np.asarray(mask, dtype=np.int32)
    Wq = np.asarray(Wq, dtype=np.float32)
    Wk = np.asarray(Wk, dtype=np.float32)
    bq = np.asarray(bq, dtype=np.float32)
    bk = np.asarray(bk, dtype=np.float32)
    v_param = np.asarray(v_param, dtype=np.float32)
    b_param = np.asarray(b_param, dtype=np.float32)

    wqt = np.ascontiguousarray(Wq.T)
    wkt = np.ascontiguousarray(Wk.T)
    bq2 = np.ascontiguousarray(bq.reshape(HC, 128).T)
    bk2 = np.ascontiguousarray(bk.reshape(HC, 128).T)
    bp2 = np.ascontiguousarray(b_param.reshape(HC, 128).T)
    vp2 = np.ascontiguousarray(v_param.reshape(HC, 128).T)
    ident = np.eye(128, dtype=np.float32)

    in_maps = []
    for b in range(B):
        in_maps.append({
            "qt": np.ascontiguousarray(Q[b].T),
            "kt": np.ascontiguousarray(K[b].T),
            "v": np.ascontiguousarray(V[b]),
            "wqt": wqt,
            "wkt": wkt,
            "bq2": bq2,
            "bk2": bk2,
            "bp2": bp2,
            "vp2": vp2,
            "mask2": np.ascontiguousarray(mask[b].reshape(1, LK)),
            "ident": ident,
        })
    return in_maps


def kernel(Q, K, V, mask, Wq, bq, Wk, bk, v_param, b_param, _trace=False):
    from concourse.bass_utils import run_bass_kernel_spmd

    nc = _get_nc()
    in_maps = make_in_maps(Q, K, V, mask, Wq, bq, Wk, bk, v_param, b_param)
    res = run_bass_kernel_spmd(nc, in_maps, core_ids=list(range(B)),
                               trace=_trace)
    outs = res.results
    context = np.stack([np.asarray(outs[b]["out_ctx"]) for b in range(B)])
    attn = np.stack([np.asarray(outs[b]["out_attn"]) for b in range(B)])
    if _trace:
        return (context, attn), res
    return context, attn
